# revision 73
# baseline (speedup 1.0000x reference)
"""Trainium2 Bass kernel for nn_Discriminator (dense MLP + pairwise L1 diversity).

Sharding: data-parallel over N=1024 rows across 8 cores (128 rows each).
The tiny M = h @ Wd + bd ([N,100]) tensor is all-gathered (bf16) per block;
each core then computes its row-block of the N x N diversity reduction:

    div[j,k] = sum_i exp( - sum_d |M[i,k,d] - M[j,k,d]| )

Engine mapping per (k,d):
  - DMA broadcasts row kd of gathered M^T to a [128,1024] bf16 tile B
  - DVE tensor_scalar: A = |B - M_own[:,kd]|  (op0=subtract, op1=abs_max, 4x mode)
  - PE: identity matmul accumulates A over d into PSUM (l1, fp32)
  - ACT: one activation(Exp, scale=-1, accum_out=...) fuses exp + sum_i,
    writing the [128,1] result directly into the concat tile's column.
"""

import os
import sys

import numpy as np

sys.path.insert(0, "/opt/trn_rl_repo")

import concourse.bass as bass
import concourse.bacc as bacc
import concourse.tile as tile
from concourse import mybir
from concourse.bass_utils import run_bass_kernel_spmd

try:
    import ml_dtypes

    BF16_NP = ml_dtypes.bfloat16
except ImportError:  # pragma: no cover
    BF16_NP = None

F32 = mybir.dt.float32
BF16 = mybir.dt.bfloat16

N = 1024
NF = 512
HID = 256
NK = 10
KD = 10
MB = NK * KD  # 100
CAT = HID + NK  # 266
EPS = 1e-3
ALPHA = 0.3
NCORES = 8
P = N // NCORES  # 128 rows per core

AF = mybir.ActivationFunctionType
ALU = mybir.AluOpType


def _chunks(total, size):
    out = []
    o = 0
    while o < total:
        out.append((o, min(size, total - o)))
        o += size
    return out


def build_program(stage="full"):
    nc = bacc.Bacc(
        "TRN2",
        target_bir_lowering=False,
        debug=False,
        num_devices=NCORES,
    )

    # ---- per-core external inputs ----
    xT = nc.dram_tensor("xT", [NF, P], F32, kind="ExternalInput")
    W0 = nc.dram_tensor("W0", [NF, HID], F32, kind="ExternalInput")
    b0c = nc.dram_tensor("b0c", [HID, 1], F32, kind="ExternalInput")
    Wd0 = nc.dram_tensor("Wd0", [HID, MB], F32, kind="ExternalInput")
    bd0c = nc.dram_tensor("bd0c", [MB, 1], F32, kind="ExternalInput")
    beta0b = nc.dram_tensor("beta0b", [P, CAT], F32, kind="ExternalInput")
    W1 = nc.dram_tensor("W1", [CAT, HID], F32, kind="ExternalInput")
    b1c = nc.dram_tensor("b1c", [HID, 1], F32, kind="ExternalInput")
    Wd1 = nc.dram_tensor("Wd1", [HID, MB], F32, kind="ExternalInput")
    bd1c = nc.dram_tensor("bd1c", [MB, 1], F32, kind="ExternalInput")
    beta1b = nc.dram_tensor("beta1b", [P, CAT], F32, kind="ExternalInput")
    Wfb = nc.dram_tensor("Wfb", [P, CAT], F32, kind="ExternalInput")
    bfc = nc.dram_tensor("bfc", [P, 1], F32, kind="ExternalInput")

    # per-core one-hot [100, 10]: column m selects M^T row 10*core + m
    Ssel = nc.dram_tensor("Ssel", [MB, NK], BF16, kind="ExternalInput")

    y_out = nc.dram_tensor("y", [P, 1], F32, kind="ExternalOutput")

    # ---- NEFF-embedded constants ----
    ident_f32 = nc.inline_tensor(np.eye(128, dtype=np.float32), name="ident_f32")
    ident_bf16 = nc.inline_tensor(
        np.eye(128).astype(BF16_NP), name="ident_bf16"
    )
    ones1_f32 = nc.inline_tensor(
        np.ones((1, 128), dtype=np.float32), name="ones1_f32"
    )
    # column sums with -0.5 scaling for the Sb rows
    nh10_c = nc.inline_tensor(
        np.full((KD, 1), -0.5).astype(BF16_NP), name="nh10"
    )
    _nh2 = np.zeros((2 * KD, 2))
    _nh2[:KD, 0] = -0.5
    _nh2[KD:, 1] = -0.5
    nh20x2_c = nc.inline_tensor(_nh2.astype(BF16_NP), name="nh20x2")

    with tile.TileContext(nc, num_cores=NCORES) as tc:
        dram = tc.alloc_tile_pool(name="dram", bufs=1, space="DRAM")
        m_loc = [dram.tile([MB, P], BF16, name=f"m_loc{b}") for b in range(2)]
        m_gath = [
            dram.tile(
                [NCORES, MB, P], BF16,
                addr_space=("Local" if stage == "nocc" else "Shared"),
                name=f"m_gath{b}",
            )
            for b in range(2)
        ]
        # rows 80..99 of M^T (kernels 8, 9) and the selected kernel rows
        mt89_dram = [dram.tile([2 * KD, N], BF16, name=f"mt89_d{b}") for b in range(2)]
        mtA_dram = [dram.tile([KD, N], BF16, name=f"mtA_d{b}") for b in range(2)]
        a2a_send = [dram.tile([NCORES, P], F32, name=f"a2a_s{b}") for b in range(2)]
        a2a_recv = [
            dram.tile([NCORES, P], F32, name=f"a2a_r{b}") for b in range(2)
        ]
        consts = tc.alloc_tile_pool(name="consts", bufs=1)
        acts = tc.alloc_tile_pool(name="acts", bufs=1)
        mtiles = tc.alloc_tile_pool(name="mtiles", bufs=2)
        bpool = tc.alloc_tile_pool(name="bpool", bufs=3)
        apool = tc.alloc_tile_pool(name="apool", bufs=6)
        epool = tc.alloc_tile_pool(name="epool", bufs=3)
        rows = tc.alloc_tile_pool(name="rows", bufs=1)
        small = tc.alloc_tile_pool(name="small", bufs=4)
        ps_small = tc.alloc_tile_pool(name="ps_small", bufs=2, space="PSUM")
        ps_l1 = tc.alloc_tile_pool(name="ps_l1", bufs=3, space="PSUM")

        # ---------- load constants ----------
        # alternate the two descriptor-generation paths for startup overlap
        _ld_flip = [0]

        def load(dram, shape, dtype=F32, name=None):
            t = consts.tile(shape, dtype, name=name)
            eng = nc.sync if _ld_flip[0] % 2 == 0 else nc.gpsimd
            _ld_flip[0] += 1
            eng.dma_start(out=t, in_=dram)
            return t

        xT_sb = [
            load(xT[o : o + sz, :], [sz, P], name=f"xT{i}")
            for i, (o, sz) in enumerate(_chunks(NF, 128))
        ]
        w0_sb = [
            load(W0[o : o + sz, :], [sz, HID], name=f"w0_{i}")
            for i, (o, sz) in enumerate(_chunks(NF, 128))
        ]
        idf = load(ident_f32[:, :], [128, 128], name="idf")
        idb = load(ident_bf16[:, :], [128, 128], BF16, name="idb")
        ones1 = load(ones1_f32[:, :], [1, 128], name="ones1")
        nh10 = load(nh10_c[:, :], [KD, 1], BF16, name="nh10")
        nh20x2 = load(nh20x2_c[:, :], [2 * KD, 2], BF16, name="nh20x2")
        ssel_sb = load(Ssel[:, :], [MB, NK], BF16, name="ssel")
        w1_sb = [
            load(W1[o : o + sz, :], [sz, HID], name=f"w1_{i}")
            for i, (o, sz) in enumerate(_chunks(CAT, 128))
        ]
        wd0_sb = [
            load(Wd0[o : o + sz, :], [sz, MB], name=f"wd0_{i}")
            for i, (o, sz) in enumerate(_chunks(HID, 128))
        ]
        wd1_sb = [
            load(Wd1[o : o + sz, :], [sz, MB], name=f"wd1_{i}")
            for i, (o, sz) in enumerate(_chunks(HID, 128))
        ]
        b0_sb = [
            load(b0c[o : o + sz, :], [sz, 1], name=f"b0_{i}")
            for i, (o, sz) in enumerate(_chunks(HID, 128))
        ]
        b1_sb = [
            load(b1c[o : o + sz, :], [sz, 1], name=f"b1_{i}")
            for i, (o, sz) in enumerate(_chunks(HID, 128))
        ]
        bd0_sb = load(bd0c[:, :], [MB, 1], name="bd0")
        bd1_sb = load(bd1c[:, :], [MB, 1], name="bd1")
        beta_sb = [
            load(beta0b[:, :], [P, CAT], name="beta0"),
            load(beta1b[:, :], [P, CAT], name="beta1"),
        ]
        wf_sb = load(Wfb[:, :], [P, CAT], name="wf")
        bf_sb = load(bfc[:, :], [P, 1], name="bf")

        eps_sb = consts.tile([P, 1], F32, name="eps")
        nc.vector.memset(eps_sb, EPS)

        # ---------- one block ----------
        def block(b, prevT, w_sb, b_sb, wd_sb, bd_sb, do_div=True, upto=None):
            """prevT: list of (tile, psize) feature-major chunks of the input.

            Returns cat tile [P, CAT] = LeakyReLU(LN(concat(h, div))).
            """
            # h^T = W^T @ prev + b   (feature-major, HID x P as 2 chunks)
            hT = []
            for mi, (mo, msz) in enumerate(_chunks(HID, 128)):
                ps = ps_small.tile([128, P], F32, tag="ps_small")
                for ki, (wt, (pt, psz)) in enumerate(zip(w_sb, prevT)):
                    nc.tensor.matmul(
                        ps[:msz, :],
                        wt[:, mo : mo + msz],
                        pt,
                        start=(ki == 0),
                        stop=(ki == len(w_sb) - 1),
                    )
                ht = acts.tile([msz, P], F32, name=f"hT{b}_{mi}")
                nc.vector.tensor_scalar(
                    out=ht, in0=ps[:msz, :], scalar1=b_sb[mi], scalar2=None,
                    op0=ALU.add,
                )
                hT.append((ht, msz))
            if upto == "h":
                return hT[0][0]

            # M^T = Wd^T @ h + bd   [100, 128]
            ps_m = ps_small.tile([MB, P], F32, tag="ps_small")
            for ki, ((ht, _), wdt) in enumerate(zip(hT, wd_sb)):
                nc.tensor.matmul(
                    ps_m,
                    wdt,
                    ht,
                    start=(ki == 0),
                    stop=(ki == len(wd_sb) - 1),
                )
            mT = mtiles.tile([MB, P], F32, tag="mT")
            nc.vector.tensor_scalar(
                out=mT, in0=ps_m, scalar1=bd_sb, scalar2=None, op0=ALU.add
            )

            # own M rows (row-major, fp32) for per-partition scalars
            ps_t = ps_small.tile([128, MB], F32, tag="ps_small")
            nc.tensor.transpose(ps_t[:, :], mT, idf[:MB, :MB])
            m_row = mtiles.tile([P, MB], F32, tag="m_row")
            nc.vector.tensor_copy(m_row, ps_t[:, :MB])
            if upto == "m":
                return m_row

            # concat tile; div columns are filled by the diversity loop
            cat = acts.tile([P, CAT], F32, name=f"cat{b}")
            if not do_div:
                nc.vector.memset(cat[:, HID:CAT], 1.0)

            # ---- gather M^T and build per-core slices ----
            # unit u=0..7: (kernel = sel-core, J-block = u)
            # unit u=8, 9: (kernel 8/9, J-block = own rows)
            if do_div:
                mT_bf = mtiles.tile([MB, P], BF16, tag="mT_bf")
                nc.vector.tensor_copy(mT_bf, mT)
                nc.gpsimd.dma_start(out=m_loc[b][:, :], in_=mT_bf)
                if stage == "nocc":
                    for c in range(NCORES):
                        nc.sync.dma_start(
                            out=m_gath[b][c, :, :], in_=m_loc[b][:, :]
                        )
                else:
                    nc.gpsimd.collective_compute(
                        "AllGather",
                        ALU.bypass,
                        replica_groups=[list(range(NCORES))],
                        ins=[m_loc[b][:, :]],
                        outs=[m_gath[b][:, :, :]],
                    )
                # one DMA assembles [100, 1024] from the gathered blocks
                mt_sb = mtiles.tile([MB, N], BF16, tag="mt_sb")
                gsrc = m_gath[b][:, :, :]
                gath_ap = bass.AP(
                    tensor=gsrc.tensor,
                    offset=gsrc.offset,
                    ap=[[P, MB], [MB * P, NCORES], [1, P]],
                )
                nc.gpsimd.dma_start(out=mt_sb, in_=gath_ap)
                # kernels 8,9 rows -> DRAM (for broadcast) and base-0 SBUF
                nc.gpsimd.dma_start(out=mt89_dram[b][:, :], in_=mt_sb[80:100, :])
                mt89_sb = mtiles.tile([2 * KD, N], BF16, tag="mt89_sb")
                nc.gpsimd.dma_start(out=mt89_sb, in_=mt89_dram[b][:, :])
                # same 20 rows flattened onto partition 0 (partition_broadcast
                # sources must start at partition 0)
                mt89_row = rows.tile([1, 2 * KD * N], BF16, tag="mt89_row")
                nc.gpsimd.dma_start(
                    out=mt89_row,
                    in_=bass.AP(
                        tensor=mt89_dram[b][:, :].tensor,
                        offset=mt89_dram[b][:, :].offset,
                        ap=[[0, 1], [1, 2 * KD * N]],
                    ),
                )
                # own kernel's rows: one-hot select -> [10, N] -> DRAM
                mtA_sb = mtiles.tile([KD, N], BF16, tag="mtA_sb")
                for ho, hsz in _chunks(N, 512):
                    ps_sel = ps_small.tile([KD, 512], F32, tag="ps_small")
                    nc.tensor.matmul(
                        ps_sel[:, :hsz], ssel_sb, mt_sb[:, ho : ho + hsz],
                        start=True, stop=True,
                    )
                    nc.scalar.activation(
                        mtA_sb[:, ho : ho + hsz], ps_sel[:, :hsz], AF.Copy,
                        bias=0.0, scale=1.0,
                    )
                nc.gpsimd.dma_start(out=mtA_dram[b][:, :], in_=mtA_sb)

                # -Sb/2 rows ([1, N] fp32 at partition 0) for the 3 kernels
                def sbrow(lhsT, rhs_sb, nm):
                    row = rows.tile([1, N], F32, tag=nm)
                    for ho, hsz in _chunks(N, 512):
                        ps_r = ps_small.tile([1, 512], F32, tag="ps_small")
                        nc.tensor.matmul(
                            ps_r[:, :hsz], lhsT, rhs_sb[:, ho : ho + hsz],
                            start=True, stop=True,
                        )
                        nc.scalar.activation(
                            row[:, ho : ho + hsz], ps_r[:, :hsz], AF.Copy,
                            bias=0.0, scale=1.0,
                        )
                    return row

                negsbA = sbrow(nh10, mtA_sb, "negsbA")
                negsb8 = sbrow(nh20x2[:, 0:1], mt89_sb, "negsb8")
                negsb9 = sbrow(nh20x2[:, 1:2], mt89_sb, "negsb9")

                # broadcast mega-tiles [128, 10*N]: same row set on every
                # partition (DMA reads the DRAM rows 128 times)
                def bmega(dram_ap, nm):
                    bt = bpool.tile([P, KD * N], BF16, tag="bt")
                    bcast = bass.AP(
                        tensor=dram_ap.tensor,
                        offset=dram_ap.offset,
                        ap=[[0, P], [1, KD * N]],
                    )
                    nc.gpsimd.dma_start(out=bt, in_=bcast)
                    return bt

                btA = bmega(mtA_dram[b][0:1, :], "btA")

                # kernels 8/9: broadcast on the (otherwise idle) Pool engine
                # straight from SBUF; needed only at the end of the unit loop
                def bmega_pool(row0):
                    bt = bpool.tile([P, KD * N], BF16, tag="bt")
                    for d in range(KD):
                        nc.gpsimd.partition_broadcast(
                            bt[:, d * N : (d + 1) * N],
                            mt89_row[0:1, (row0 + d) * N : (row0 + d + 1) * N],
                        )
                    return bt

                bt8 = bmega_pool(0)
                bt9 = bmega_pool(KD)

                divsend = acts.tile([P, NCORES], F32, name=f"divsend{b}")

                for u in range(NK):
                    if u < NCORES:
                        bt, negsb = btA, negsbA
                        # scalars: M[J-block u rows, own-kernel cols] via
                        # one-hot column selection
                        ps_sc = ps_small.tile([128, KD], F32, tag="ps_small")
                        nc.tensor.matmul(
                            ps_sc[:, :KD],
                            mt_sb[:, u * P : (u + 1) * P],
                            ssel_sb,
                            start=True,
                            stop=True,
                        )
                        scal = small.tile([P, KD], F32, tag="scal")
                        nc.vector.tensor_copy(scal, ps_sc[:, :KD])
                        accum_dst = divsend[:, u : u + 1]
                    else:
                        bt = bt8 if u == 8 else bt9
                        negsb = negsb8 if u == 8 else negsb9
                        scal = small.tile([P, KD], F32, tag="scal")
                        nc.vector.tensor_copy(
                            scal, m_row[:, (u - 8 + 8) * KD : (u - 7 + 8) * KD]
                        )
                        accum_dst = cat[:, HID + u : HID + u + 1]
                    nss = small.tile([P, 1], F32, tag="nss")
                    nc.vector.tensor_reduce(
                        out=nss, in_=scal, axis=mybir.AxisListType.X,
                        op=ALU.add, negate=True,
                    )
                    psl = ps_l1.tile([P, N], F32, tag="psl")

                    def relu_d(d):
                        at = apool.tile([P, N], BF16, tag="at")
                        nc.vector.tensor_scalar(
                            out=at,
                            in0=bt[:, d * N : (d + 1) * N],
                            scalar1=scal[:, d : d + 1],
                            scalar2=0.0,
                            op0=ALU.subtract,
                            op1=ALU.max,
                        )
                        return at

                    def stream(at, first):
                        for ho, hsz in _chunks(N, 512):
                            nc.tensor.matmul(
                                psl[:, ho : ho + hsz],
                                idb,
                                at[:, ho : ho + hsz],
                                start=first,
                                stop=False,
                            )

                    # d = 0..5 stream straight into PSUM; d = 6..9 are
                    # pre-added pairwise on DVE to offload the PE
                    for d in range(6):
                        stream(relu_d(d), d == 0)
                    for lo in (6, 8):
                        a0, a1 = relu_d(lo), relu_d(lo + 1)
                        comb = apool.tile([P, N], BF16, tag="comb")
                        nc.vector.tensor_add(comb, a0, a1)
                        stream(comb, False)
                    for ho, hsz in _chunks(N, 512):
                        nc.tensor.matmul(
                            psl[:, ho : ho + hsz],
                            ones1,
                            negsb[:, ho : ho + hsz],
                            start=False,
                            stop=True,
                        )
                    escr = epool.tile([P, N], BF16, tag="escr")
                    nc.scalar.activation(
                        escr, psl, AF.Exp, bias=nss, scale=-2.0,
                        accum_out=accum_dst,
                    )

                # exchange div columns: shard u of our send buffer holds the
                # result for core u; AllToAll routes sender k's shard c to
                # slot k on core c  ->  recv[k] = div[own rows, kernel k]
                ps_ds = ps_small.tile([128, P], F32, tag="ps_small")
                nc.tensor.transpose(ps_ds[:NCORES, :], divsend, idf)
                dsend_sb = small.tile([NCORES, P], F32, tag="dsend")
                nc.vector.tensor_copy(dsend_sb, ps_ds[:NCORES, :])
                nc.gpsimd.dma_start(out=a2a_send[b][:, :], in_=dsend_sb)
                if stage == "nocc":
                    nc.gpsimd.dma_start(
                        out=a2a_recv[b][:, :], in_=a2a_send[b][:, :]
                    )
                else:
                    nc.gpsimd.collective_compute(
                        "AllToAll",
                        ALU.bypass,
                        replica_groups=[list(range(NCORES))],
                        ins=[a2a_send[b][:, :]],
                        outs=[a2a_recv[b][:, :]],
                    )
                drecv_sb = small.tile([NCORES, P], F32, tag="drecv")
                nc.gpsimd.dma_start(out=drecv_sb, in_=a2a_recv[b][:, :])
                ps_dr = ps_small.tile([128, NCORES], F32, tag="ps_small")
                nc.tensor.transpose(
                    ps_dr[:, :NCORES], drecv_sb, idf[:NCORES, :NCORES]
                )
                nc.vector.tensor_copy(
                    cat[:, HID : HID + NCORES], ps_dr[:, :NCORES]
                )

            # h rows into cat[:, :256] via PE transposes of hT
            for mi, (ht, msz) in enumerate(hT):
                ps_t2 = ps_small.tile([128, P], F32, tag="ps_small")
                nc.tensor.transpose(ps_t2[:, :msz], ht, idf[:msz, :msz])
                nc.vector.tensor_copy(
                    cat[:, mi * 128 : mi * 128 + msz], ps_t2[:, :msz]
                )

            if upto == "cat":
                return cat
            # LayerNorm (center+scale, beta only) + LeakyReLU
            stats = small.tile([P, 6], F32, tag="stats")
            nc.vector.bn_stats(out=stats, in_=cat)
            mv = small.tile([P, 2], F32, tag="mv")
            nc.vector.bn_aggr(out=mv, in_=stats)
            rstd = small.tile([P, 1], F32, tag="rstd")
            nc.scalar.activation(
                rstd, mv[:, 1:2], AF.Sqrt, bias=eps_sb, scale=1.0
            )
            nc.vector.reciprocal(out=rstd, in_=rstd)
            if upto == "stats":
                return mv
            catn = acts.tile([P, CAT], F32, name=f"catn{b}")
            nc.vector.tensor_scalar(
                out=catn,
                in0=cat,
                scalar1=mv[:, 0:1],
                scalar2=rstd,
                op0=ALU.subtract,
                op1=ALU.mult,
            )
            nc.vector.tensor_add(catn, catn, beta_sb[b])
            if upto == "ln":
                return catn
            # leaky relu: max(x, 0.3x)
            scr = acts.tile([P, CAT], F32, name=f"lrelu{b}")
            nc.scalar.activation(scr, catn, AF.Copy, bias=0.0, scale=ALPHA)
            hout = acts.tile([P, CAT], F32, name=f"hout{b}")
            nc.vector.tensor_tensor(
                out=hout, in0=catn, in1=scr, op=ALU.max
            )
            if upto == "lrelu":
                return hout
            return hout

        # ---------- block 0 ----------
        prev0 = [(t, 128) for t in xT_sb]
        upto = stage if stage in ("h", "m", "cat", "stats", "ln", "lrelu") else None
        h1 = block(0, prev0, w0_sb, b0_sb, wd0_sb, bd0_sb,
                   do_div=(stage in ("full", "b0", "nocc")), upto=upto)
        if upto is not None:
            ytmp = small.tile([P, 1], F32, tag="ysb")
            nc.vector.tensor_copy(ytmp, h1[:, 0:1])
            nc.sync.dma_start(out=y_out[:, :], in_=ytmp)
            h1 = None

        if upto is not None:
            pass
        elif stage in ("full", "nocc"):
            # transpose h1 -> feature-major chunks for block 1
            h1T = []
            for ci, (co, csz) in enumerate(_chunks(CAT, 128)):
                ps_t = ps_small.tile([128, P], F32, tag="ps_small")
                nc.tensor.transpose(ps_t[:csz, :], h1[:, co : co + csz], idf)
                ht = acts.tile([csz, P], F32, name=f"h1T_{ci}")
                nc.vector.tensor_copy(ht, ps_t[:csz, :])
                h1T.append((ht, csz))

            # ---------- block 1 ----------
            h2 = block(1, h1T, w1_sb, b1_sb, wd1_sb, bd1_sb)
        else:
            h2 = h1

        # ---------- critic head: y = h2 @ Wf + bf ----------
        if upto is None:
            hw = acts.tile([P, CAT], F32, name="hw")
            yacc = small.tile([P, 1], F32, tag="yacc")
            nc.vector.tensor_mul(hw, h2, wf_sb)
            nc.vector.tensor_reduce(
                out=yacc, in_=hw, axis=mybir.AxisListType.X, op=ALU.add
            )
            ysb = small.tile([P, 1], F32, tag="ysb")
            nc.scalar.activation(ysb, yacc, AF.Identity, bias=bf_sb, scale=1.0)
            nc.sync.dma_start(out=y_out[:, :], in_=ysb)

        ps_l1.release()
        ps_small.release()
        small.release()
        rows.release()
        epool.release()
        apool.release()
        bpool.release()
        mtiles.release()
        acts.release()
        consts.release()
        dram.release()

    nc.compile()
    return nc


_NC_CACHE = {}


def _get_nc():
    stage = os.environ.get("KERNEL_STAGE", "full")
    if stage not in _NC_CACHE:
        _NC_CACHE[stage] = build_program(stage)
    return _NC_CACHE[stage]


def _make_in_maps(inputs):
    f = lambda a: np.ascontiguousarray(np.asarray(a, dtype=np.float32))
    x = f(inputs["x"])
    shared = {
        "W0": f(inputs["W0"]),
        "b0c": f(inputs["b0"]).reshape(HID, 1),
        "Wd0": f(inputs["Wd0"]),
        "bd0c": f(inputs["bd0"]).reshape(MB, 1),
        "beta0b": np.ascontiguousarray(
            np.broadcast_to(f(inputs["beta0"]), (P, CAT))
        ),
        "W1": f(inputs["W1"]),
        "b1c": f(inputs["b1"]).reshape(HID, 1),
        "Wd1": f(inputs["Wd1"]),
        "bd1c": f(inputs["bd1"]).reshape(MB, 1),
        "beta1b": np.ascontiguousarray(
            np.broadcast_to(f(inputs["beta1"]), (P, CAT))
        ),
        "Wfb": np.ascontiguousarray(
            np.broadcast_to(f(inputs["Wf"]).reshape(1, CAT), (P, CAT))
        ),
        "bfc": np.full((P, 1), float(np.asarray(inputs["bf"]).reshape(-1)[0]),
                       dtype=np.float32),
    }
    if BF16_NP is None:
        raise RuntimeError("ml_dtypes required for bf16 inputs")
    in_maps = []
    for c in range(NCORES):
        m = dict(shared)
        m["xT"] = np.ascontiguousarray(x[c * P : (c + 1) * P, :].T)
        sel = np.zeros((MB, NK), dtype=np.float32)
        for j in range(NK):
            sel[(10 * c + j) % MB, j] = 1.0
        m["Ssel"] = sel.astype(BF16_NP)
        in_maps.append(m)
    return in_maps


def run(inputs, **kw):
    nc = _get_nc()
    in_maps = _make_in_maps(inputs)
    res = run_bass_kernel_spmd(nc, in_maps, list(range(NCORES)), **kw)
    y = np.concatenate([res.results[c]["y"] for c in range(NCORES)], axis=0)
    return y.astype(np.float32), res


def kernel(**inputs) -> np.ndarray:
    y, _ = run(inputs)
    return y


# revision 87
# speedup vs baseline: 1.0396x; 1.0396x over previous
"""Trainium2 Bass kernel for nn_Discriminator (dense MLP + pairwise L1 diversity).

Sharding: data-parallel over N=1024 rows across 8 cores (128 rows each).
The tiny M = h @ Wd + bd ([N,100]) tensor is all-gathered (bf16) per block;
each core then computes its row-block of the N x N diversity reduction:

    div[j,k] = sum_i exp( - sum_d |M[i,k,d] - M[j,k,d]| )

Engine mapping per (k,d):
  - DMA broadcasts row kd of gathered M^T to a [128,1024] bf16 tile B
  - DVE tensor_scalar: A = |B - M_own[:,kd]|  (op0=subtract, op1=abs_max, 4x mode)
  - PE: identity matmul accumulates A over d into PSUM (l1, fp32)
  - ACT: one activation(Exp, scale=-1, accum_out=...) fuses exp + sum_i,
    writing the [128,1] result directly into the concat tile's column.
"""

import os
import sys

import numpy as np

sys.path.insert(0, "/opt/trn_rl_repo")

import concourse.bass as bass
import concourse.bacc as bacc
import concourse.tile as tile
from concourse import mybir
from concourse.bass_utils import run_bass_kernel_spmd

try:
    import ml_dtypes

    BF16_NP = ml_dtypes.bfloat16
except ImportError:  # pragma: no cover
    BF16_NP = None

F32 = mybir.dt.float32
BF16 = mybir.dt.bfloat16

N = 1024
NF = 512
HID = 256
NK = 10
KD = 10
MB = NK * KD  # 100
CAT = HID + NK  # 266
EPS = 1e-3
ALPHA = 0.3
NCORES = 8
P = N // NCORES  # 128 rows per core

AF = mybir.ActivationFunctionType
ALU = mybir.AluOpType


def _chunks(total, size):
    out = []
    o = 0
    while o < total:
        out.append((o, min(size, total - o)))
        o += size
    return out


def build_program(stage="full"):
    nc = bacc.Bacc(
        "TRN2",
        target_bir_lowering=False,
        debug=False,
        num_devices=NCORES,
    )

    # ---- per-core external inputs ----
    xT = nc.dram_tensor("xT", [NF, P], F32, kind="ExternalInput")
    W0 = nc.dram_tensor("W0", [NF, HID], F32, kind="ExternalInput")
    b0c = nc.dram_tensor("b0c", [HID, 1], F32, kind="ExternalInput")
    Wd0 = nc.dram_tensor("Wd0", [HID, MB], F32, kind="ExternalInput")
    bd0c = nc.dram_tensor("bd0c", [MB, 1], F32, kind="ExternalInput")
    beta0b = nc.dram_tensor("beta0b", [P, CAT], F32, kind="ExternalInput")
    W1 = nc.dram_tensor("W1", [CAT, HID], F32, kind="ExternalInput")
    b1c = nc.dram_tensor("b1c", [HID, 1], F32, kind="ExternalInput")
    Wd1 = nc.dram_tensor("Wd1", [HID, MB], F32, kind="ExternalInput")
    bd1c = nc.dram_tensor("bd1c", [MB, 1], F32, kind="ExternalInput")
    beta1b = nc.dram_tensor("beta1b", [P, CAT], F32, kind="ExternalInput")
    Wfb = nc.dram_tensor("Wfb", [P, CAT], F32, kind="ExternalInput")
    bfc = nc.dram_tensor("bfc", [P, 1], F32, kind="ExternalInput")

    # per-core one-hot [100, 10]: column m selects M^T row 10*core + m
    Ssel = nc.dram_tensor("Ssel", [MB, NK], BF16, kind="ExternalInput")

    y_out = nc.dram_tensor("y", [P, 1], F32, kind="ExternalOutput")

    # ---- NEFF-embedded constants ----
    ident_f32 = nc.inline_tensor(np.eye(128, dtype=np.float32), name="ident_f32")
    ident_bf16 = nc.inline_tensor(
        np.eye(128).astype(BF16_NP), name="ident_bf16"
    )
    ones1_f32 = nc.inline_tensor(
        np.ones((1, 128), dtype=np.float32), name="ones1_f32"
    )
    # column sums with -0.5 scaling for the Sb rows
    nh10_c = nc.inline_tensor(
        np.full((KD, 1), -0.5).astype(BF16_NP), name="nh10"
    )
    _nh2 = np.zeros((2 * KD, 2))
    _nh2[:KD, 0] = -0.5
    _nh2[KD:, 1] = -0.5
    nh20x2_c = nc.inline_tensor(_nh2.astype(BF16_NP), name="nh20x2")

    with tile.TileContext(nc, num_cores=NCORES) as tc:
        dram = tc.alloc_tile_pool(name="dram", bufs=1, space="DRAM")
        m_loc = [dram.tile([MB, P], BF16, name=f"m_loc{b}") for b in range(2)]
        m_gath = [
            dram.tile(
                [NCORES, MB, P], BF16,
                addr_space=("Local" if stage == "nocc" else "Shared"),
                name=f"m_gath{b}",
            )
            for b in range(2)
        ]
        # rows 80..99 of M^T (kernels 8, 9) and the selected kernel rows
        mt89_dram = [dram.tile([2 * KD, N], BF16, name=f"mt89_d{b}") for b in range(2)]
        mtA_dram = [dram.tile([KD, N], BF16, name=f"mtA_d{b}") for b in range(2)]
        # AllToAll of M^T rows 0..79: shard c = rows of kernel c, so every
        # core receives its own kernel's rows from all peers (1/10th the
        # AllGather payload, and off the mt_sb assembly path)
        mtam_recv = [
            dram.tile([NCORES, KD, P], BF16, name=f"mtam_r{b}") for b in range(2)
        ]
        a2a_send = [dram.tile([NCORES, P], F32, name=f"a2a_s{b}") for b in range(2)]
        a2a_recv = [
            dram.tile([NCORES, P], F32, name=f"a2a_r{b}") for b in range(2)
        ]
        consts = tc.alloc_tile_pool(name="consts", bufs=1)
        acts = tc.alloc_tile_pool(name="acts", bufs=1)
        mtiles = tc.alloc_tile_pool(name="mtiles", bufs=2)
        bpool = tc.alloc_tile_pool(name="bpool", bufs=2)
        apool = tc.alloc_tile_pool(name="apool", bufs=6)
        epool = tc.alloc_tile_pool(name="epool", bufs=2)
        rows = tc.alloc_tile_pool(name="rows", bufs=1)
        small = tc.alloc_tile_pool(name="small", bufs=4)
        ps_small = tc.alloc_tile_pool(name="ps_small", bufs=2, space="PSUM")
        ps_l1 = tc.alloc_tile_pool(name="ps_l1", bufs=3, space="PSUM")

        # ---------- load constants ----------
        # alternate the two descriptor-generation paths for startup overlap
        _ld_flip = [0]

        def load(dram, shape, dtype=F32, name=None):
            t = consts.tile(shape, dtype, name=name)
            eng = nc.sync if _ld_flip[0] % 2 == 0 else nc.gpsimd
            _ld_flip[0] += 1
            eng.dma_start(out=t, in_=dram)
            return t

        xT_sb = [
            load(xT[o : o + sz, :], [sz, P], name=f"xT{i}")
            for i, (o, sz) in enumerate(_chunks(NF, 128))
        ]
        w0_sb = [
            load(W0[o : o + sz, :], [sz, HID], name=f"w0_{i}")
            for i, (o, sz) in enumerate(_chunks(NF, 128))
        ]
        idf = load(ident_f32[:, :], [128, 128], name="idf")
        idb = load(ident_bf16[:, :], [128, 128], BF16, name="idb")
        ones1 = load(ones1_f32[:, :], [1, 128], name="ones1")
        nh10 = load(nh10_c[:, :], [KD, 1], BF16, name="nh10")
        nh20x2 = load(nh20x2_c[:, :], [2 * KD, 2], BF16, name="nh20x2")
        w1_sb = [
            load(W1[o : o + sz, :], [sz, HID], name=f"w1_{i}")
            for i, (o, sz) in enumerate(_chunks(CAT, 128))
        ]
        wd0_sb = [
            load(Wd0[o : o + sz, :], [sz, MB], name=f"wd0_{i}")
            for i, (o, sz) in enumerate(_chunks(HID, 128))
        ]
        wd1_sb = [
            load(Wd1[o : o + sz, :], [sz, MB], name=f"wd1_{i}")
            for i, (o, sz) in enumerate(_chunks(HID, 128))
        ]
        b0_sb = [
            load(b0c[o : o + sz, :], [sz, 1], name=f"b0_{i}")
            for i, (o, sz) in enumerate(_chunks(HID, 128))
        ]
        b1_sb = [
            load(b1c[o : o + sz, :], [sz, 1], name=f"b1_{i}")
            for i, (o, sz) in enumerate(_chunks(HID, 128))
        ]
        bd0_sb = load(bd0c[:, :], [MB, 1], name="bd0")
        bd1_sb = load(bd1c[:, :], [MB, 1], name="bd1")
        beta_sb = [
            load(beta0b[:, :], [P, CAT], name="beta0"),
            load(beta1b[:, :], [P, CAT], name="beta1"),
        ]
        wf_sb = load(Wfb[:, :], [P, CAT], name="wf")
        bf_sb = load(bfc[:, :], [P, 1], name="bf")

        eps_sb = consts.tile([P, 1], F32, name="eps")
        nc.vector.memset(eps_sb, EPS)

        # ---------- one block ----------
        def block(b, prevT, w_sb, b_sb, wd_sb, bd_sb, do_div=True, upto=None):
            """prevT: list of (tile, psize) feature-major chunks of the input.

            Returns cat tile [P, CAT] = LeakyReLU(LN(concat(h, div))).
            """
            # h^T = W^T @ prev + b   (feature-major, HID x P as 2 chunks)
            hT = []
            for mi, (mo, msz) in enumerate(_chunks(HID, 128)):
                ps = ps_small.tile([128, P], F32, tag="ps_small")
                for ki, (wt, (pt, psz)) in enumerate(zip(w_sb, prevT)):
                    nc.tensor.matmul(
                        ps[:msz, :],
                        wt[:, mo : mo + msz],
                        pt,
                        start=(ki == 0),
                        stop=(ki == len(w_sb) - 1),
                    )
                ht = acts.tile([msz, P], F32, name=f"hT{b}_{mi}")
                nc.vector.tensor_scalar(
                    out=ht, in0=ps[:msz, :], scalar1=b_sb[mi], scalar2=None,
                    op0=ALU.add,
                )
                hT.append((ht, msz))
            if upto == "h":
                return hT[0][0]

            # M^T = Wd^T @ h + bd   [100, 128]
            ps_m = ps_small.tile([MB, P], F32, tag="ps_small")
            for ki, ((ht, _), wdt) in enumerate(zip(hT, wd_sb)):
                nc.tensor.matmul(
                    ps_m,
                    wdt,
                    ht,
                    start=(ki == 0),
                    stop=(ki == len(wd_sb) - 1),
                )
            mT = mtiles.tile([MB, P], F32, tag="mT")
            nc.vector.tensor_scalar(
                out=mT, in0=ps_m, scalar1=bd_sb, scalar2=None, op0=ALU.add
            )

            # own M rows (row-major, fp32) for per-partition scalars
            ps_t = ps_small.tile([128, MB], F32, tag="ps_small")
            nc.tensor.transpose(ps_t[:, :], mT, idf[:MB, :MB])
            m_row = mtiles.tile([P, MB], F32, tag="m_row")
            nc.vector.tensor_copy(m_row, ps_t[:, :MB])
            if upto == "m":
                return m_row

            # concat tile; div columns are filled by the diversity loop
            cat = acts.tile([P, CAT], F32, name=f"cat{b}")
            if not do_div:
                nc.vector.memset(cat[:, HID:CAT], 1.0)

            # ---- gather M^T and build per-core slices ----
            # unit u=0..7: (kernel = sel-core, J-block = u)
            # unit u=8, 9: (kernel 8/9, J-block = own rows)
            if do_div:
                mT_bf = mtiles.tile([MB, P], BF16, tag="mT_bf")
                nc.vector.tensor_copy(mT_bf, mT)
                nc.gpsimd.dma_start(out=m_loc[b][:, :], in_=mT_bf)
                if stage == "nocc":
                    nc.gpsimd.dma_start(
                        out=mtam_recv[b][:, :, :], in_=m_loc[b][0:80, :]
                    )
                    for c in range(NCORES):
                        nc.sync.dma_start(
                            out=m_gath[b][c, :, :], in_=m_loc[b][:, :]
                        )
                else:
                    nc.gpsimd.collective_compute(
                        "AllToAll",
                        ALU.bypass,
                        replica_groups=[list(range(NCORES))],
                        ins=[m_loc[b][0:80, :]],
                        outs=[mtam_recv[b][:, :, :]],
                    )
                    nc.gpsimd.collective_compute(
                        "AllGather",
                        ALU.bypass,
                        replica_groups=[list(range(NCORES))],
                        ins=[m_loc[b][:, :]],
                        outs=[m_gath[b][:, :, :]],
                    )
                # one DMA assembles [100, 1024] from the gathered blocks
                mt_sb = mtiles.tile([MB, N], BF16, tag="mt_sb")
                gsrc = m_gath[b][:, :, :]
                gath_ap = bass.AP(
                    tensor=gsrc.tensor,
                    offset=gsrc.offset,
                    ap=[[P, MB], [MB * P, NCORES], [1, P]],
                )
                nc.gpsimd.dma_start(out=mt_sb, in_=gath_ap)
                # kernels 8,9 rows -> DRAM (for broadcast) and base-0 SBUF
                nc.gpsimd.dma_start(out=mt89_dram[b][:, :], in_=mt_sb[80:100, :])
                mt89_sb = mtiles.tile([2 * KD, N], BF16, tag="mt89_sb")
                nc.gpsimd.dma_start(out=mt89_sb, in_=mt89_dram[b][:, :])
                # same 20 rows flattened onto partition 0 (partition_broadcast
                # sources must start at partition 0)
                mt89_row = rows.tile([1, 2 * KD * N], BF16, tag="mt89_row")
                nc.gpsimd.dma_start(
                    out=mt89_row,
                    in_=bass.AP(
                        tensor=mt89_dram[b][:, :].tensor,
                        offset=mt89_dram[b][:, :].offset,
                        ap=[[0, 1], [1, 2 * KD * N]],
                    ),
                )
                # own kernel's rows, assembled from the AllToAll result
                mtA_sb = mtiles.tile([KD, N], BF16, tag="mtA_sb")
                rsrc = mtam_recv[b][:, :, :]
                nc.gpsimd.dma_start(
                    out=mtA_sb,
                    in_=bass.AP(
                        tensor=rsrc.tensor,
                        offset=rsrc.offset,
                        ap=[[P, KD], [KD * P, NCORES], [1, P]],
                    ),
                )
                nc.gpsimd.dma_start(out=mtA_dram[b][:, :], in_=mtA_sb)

                # -Sb/2 rows ([1, N] fp32 at partition 0) for the 3 kernels
                def sbrow(lhsT, rhs_sb, nm):
                    row = rows.tile([1, N], F32, tag=nm)
                    for ho, hsz in _chunks(N, 512):
                        ps_r = ps_small.tile([1, 512], F32, tag="ps_small")
                        nc.tensor.matmul(
                            ps_r[:, :hsz], lhsT, rhs_sb[:, ho : ho + hsz],
                            start=True, stop=True,
                        )
                        nc.scalar.activation(
                            row[:, ho : ho + hsz], ps_r[:, :hsz], AF.Copy,
                            bias=0.0, scale=1.0,
                        )
                    return row

                negsbA = sbrow(nh10, mtA_sb, "negsbA")
                negsb8 = sbrow(nh20x2[:, 0:1], mt89_sb, "negsb8")
                negsb9 = sbrow(nh20x2[:, 1:2], mt89_sb, "negsb9")

                # broadcast mega-tiles [128, 10*N]: same row set on every
                # partition (DMA reads the DRAM rows 128 times)
                def bmega(dram_ap, nm):
                    bt = bpool.tile([P, KD * N], BF16, tag="bt")
                    bcast = bass.AP(
                        tensor=dram_ap.tensor,
                        offset=dram_ap.offset,
                        ap=[[0, P], [1, KD * N]],
                    )
                    nc.gpsimd.dma_start(out=bt, in_=bcast)
                    return bt

                # split the broadcast so unit 0 can start after the first
                # two d-slices land instead of the full 2.5 MB
                btA0 = bpool.tile([P, 2 * N], BF16, tag="btA0")
                src0 = mtA_dram[b][0:1, :]
                nc.gpsimd.dma_start(
                    out=btA0,
                    in_=bass.AP(
                        tensor=src0.tensor, offset=src0.offset,
                        ap=[[0, P], [1, 2 * N]],
                    ),
                )
                btA1 = bpool.tile([P, (KD - 2) * N], BF16, tag="btA1")
                src1 = mtA_dram[b][2:3, :]
                nc.gpsimd.dma_start(
                    out=btA1,
                    in_=bass.AP(
                        tensor=src1.tensor, offset=src1.offset,
                        ap=[[0, P], [1, (KD - 2) * N]],
                    ),
                )

                # kernels 8/9: broadcast on the (otherwise idle) Pool engine
                # straight from SBUF; needed only at the end of the unit loop
                def bmega_pool(row0):
                    bt = bpool.tile([P, KD * N], BF16, tag="bt")
                    for d in range(KD):
                        nc.gpsimd.partition_broadcast(
                            bt[:, d * N : (d + 1) * N],
                            mt89_row[0:1, (row0 + d) * N : (row0 + d + 1) * N],
                        )
                    return bt

                bt8 = bmega_pool(0)
                bt9 = bmega_pool(KD)

                divsend = acts.tile([P, NCORES], F32, name=f"divsend{b}")

                for u in range(NK):
                    if u < NCORES:
                        negsb = negsbA

                        def bt_slice(d):
                            if d < 2:
                                return btA0[:, d * N : (d + 1) * N]
                            return btA1[:, (d - 2) * N : (d - 1) * N]
                        # scalars: M[J-block u rows, own-kernel cols] =
                        # transpose of the mtA slice for block u
                        ps_sc = ps_small.tile([128, KD], BF16, tag="ps_small")
                        nc.tensor.transpose(
                            ps_sc[:, :KD],
                            mtA_sb[:, u * P : (u + 1) * P],
                            idb[:KD, :KD],
                        )
                        scal = small.tile([P, KD], F32, tag="scal")
                        nc.vector.tensor_copy(scal, ps_sc[:, :KD])
                        accum_dst = divsend[:, u : u + 1]
                    else:
                        bt = bt8 if u == 8 else bt9
                        negsb = negsb8 if u == 8 else negsb9

                        def bt_slice(d, _bt=bt):
                            return _bt[:, d * N : (d + 1) * N]
                        scal = small.tile([P, KD], F32, tag="scal")
                        nc.vector.tensor_copy(
                            scal, m_row[:, (u - 8 + 8) * KD : (u - 7 + 8) * KD]
                        )
                        accum_dst = cat[:, HID + u : HID + u + 1]
                    nss = small.tile([P, 1], F32, tag="nss")
                    nc.vector.tensor_reduce(
                        out=nss, in_=scal, axis=mybir.AxisListType.X,
                        op=ALU.add, negate=True,
                    )
                    psl = ps_l1.tile([P, N], F32, tag="psl")

                    def relu_d(d):
                        at = apool.tile([P, N], BF16, tag="at")
                        nc.vector.tensor_scalar(
                            out=at,
                            in0=bt_slice(d),
                            scalar1=scal[:, d : d + 1],
                            scalar2=0.0,
                            op0=ALU.subtract,
                            op1=ALU.max,
                        )
                        return at

                    def stream(at, first):
                        for ho, hsz in _chunks(N, 512):
                            nc.tensor.matmul(
                                psl[:, ho : ho + hsz],
                                idb,
                                at[:, ho : ho + hsz],
                                start=first,
                                stop=False,
                            )

                    # d = 0..5 stream straight into PSUM; d = 6..9 are
                    # pre-added pairwise on DVE to offload the PE
                    for d in range(6):
                        stream(relu_d(d), d == 0)
                    for lo in (6, 8):
                        a0, a1 = relu_d(lo), relu_d(lo + 1)
                        comb = apool.tile([P, N], BF16, tag="comb")
                        nc.vector.tensor_add(comb, a0, a1)
                        stream(comb, False)
                    for ho, hsz in _chunks(N, 512):
                        nc.tensor.matmul(
                            psl[:, ho : ho + hsz],
                            ones1,
                            negsb[:, ho : ho + hsz],
                            start=False,
                            stop=True,
                        )
                    escr = epool.tile([P, N], BF16, tag="escr")
                    nc.scalar.activation(
                        escr, psl, AF.Exp, bias=nss, scale=-2.0,
                        accum_out=accum_dst,
                    )

                # exchange div columns: shard u of our send buffer holds the
                # result for core u; AllToAll routes sender k's shard c to
                # slot k on core c  ->  recv[k] = div[own rows, kernel k]
                ps_ds = ps_small.tile([128, P], F32, tag="ps_small")
                nc.tensor.transpose(ps_ds[:NCORES, :], divsend, idf)
                dsend_sb = small.tile([NCORES, P], F32, tag="dsend")
                nc.vector.tensor_copy(dsend_sb, ps_ds[:NCORES, :])
                nc.gpsimd.dma_start(out=a2a_send[b][:, :], in_=dsend_sb)
                if stage == "nocc":
                    nc.gpsimd.dma_start(
                        out=a2a_recv[b][:, :], in_=a2a_send[b][:, :]
                    )
                else:
                    nc.gpsimd.collective_compute(
                        "AllToAll",
                        ALU.bypass,
                        replica_groups=[list(range(NCORES))],
                        ins=[a2a_send[b][:, :]],
                        outs=[a2a_recv[b][:, :]],
                    )
                drecv_sb = small.tile([NCORES, P], F32, tag="drecv")
                nc.gpsimd.dma_start(out=drecv_sb, in_=a2a_recv[b][:, :])
                ps_dr = ps_small.tile([128, NCORES], F32, tag="ps_small")
                nc.tensor.transpose(
                    ps_dr[:, :NCORES], drecv_sb, idf[:NCORES, :NCORES]
                )
                nc.vector.tensor_copy(
                    cat[:, HID : HID + NCORES], ps_dr[:, :NCORES]
                )

            # h rows into cat[:, :256] via PE transposes of hT
            for mi, (ht, msz) in enumerate(hT):
                ps_t2 = ps_small.tile([128, P], F32, tag="ps_small")
                nc.tensor.transpose(ps_t2[:, :msz], ht, idf[:msz, :msz])
                nc.vector.tensor_copy(
                    cat[:, mi * 128 : mi * 128 + msz], ps_t2[:, :msz]
                )

            if upto == "cat":
                return cat
            # LayerNorm (center+scale, beta only) + LeakyReLU
            stats = small.tile([P, 6], F32, tag="stats")
            nc.vector.bn_stats(out=stats, in_=cat)
            mv = small.tile([P, 2], F32, tag="mv")
            nc.vector.bn_aggr(out=mv, in_=stats)
            rstd = small.tile([P, 1], F32, tag="rstd")
            nc.scalar.activation(
                rstd, mv[:, 1:2], AF.Sqrt, bias=eps_sb, scale=1.0
            )
            nc.vector.reciprocal(out=rstd, in_=rstd)
            if upto == "stats":
                return mv
            catn = acts.tile([P, CAT], F32, name=f"catn{b}")
            nc.vector.tensor_scalar(
                out=catn,
                in0=cat,
                scalar1=mv[:, 0:1],
                scalar2=rstd,
                op0=ALU.subtract,
                op1=ALU.mult,
            )
            nc.vector.tensor_add(catn, catn, beta_sb[b])
            if upto == "ln":
                return catn
            # leaky relu: max(x, 0.3x)
            scr = acts.tile([P, CAT], F32, name=f"lrelu{b}")
            nc.scalar.activation(scr, catn, AF.Copy, bias=0.0, scale=ALPHA)
            hout = acts.tile([P, CAT], F32, name=f"hout{b}")
            nc.vector.tensor_tensor(
                out=hout, in0=catn, in1=scr, op=ALU.max
            )
            if upto == "lrelu":
                return hout
            return hout

        # ---------- block 0 ----------
        prev0 = [(t, 128) for t in xT_sb]
        upto = stage if stage in ("h", "m", "cat", "stats", "ln", "lrelu") else None
        h1 = block(0, prev0, w0_sb, b0_sb, wd0_sb, bd0_sb,
                   do_div=(stage in ("full", "b0", "nocc")), upto=upto)
        if upto is not None:
            ytmp = small.tile([P, 1], F32, tag="ysb")
            nc.vector.tensor_copy(ytmp, h1[:, 0:1])
            nc.sync.dma_start(out=y_out[:, :], in_=ytmp)
            h1 = None

        if upto is not None:
            pass
        elif stage in ("full", "nocc"):
            # transpose h1 -> feature-major chunks for block 1
            h1T = []
            for ci, (co, csz) in enumerate(_chunks(CAT, 128)):
                ps_t = ps_small.tile([128, P], F32, tag="ps_small")
                nc.tensor.transpose(ps_t[:csz, :], h1[:, co : co + csz], idf)
                ht = acts.tile([csz, P], F32, name=f"h1T_{ci}")
                nc.vector.tensor_copy(ht, ps_t[:csz, :])
                h1T.append((ht, csz))

            # ---------- block 1 ----------
            h2 = block(1, h1T, w1_sb, b1_sb, wd1_sb, bd1_sb)
        else:
            h2 = h1

        # ---------- critic head: y = h2 @ Wf + bf ----------
        if upto is None:
            hw = acts.tile([P, CAT], F32, name="hw")
            yacc = small.tile([P, 1], F32, tag="yacc")
            nc.vector.tensor_mul(hw, h2, wf_sb)
            nc.vector.tensor_reduce(
                out=yacc, in_=hw, axis=mybir.AxisListType.X, op=ALU.add
            )
            ysb = small.tile([P, 1], F32, tag="ysb")
            nc.scalar.activation(ysb, yacc, AF.Identity, bias=bf_sb, scale=1.0)
            nc.sync.dma_start(out=y_out[:, :], in_=ysb)

        ps_l1.release()
        ps_small.release()
        small.release()
        rows.release()
        epool.release()
        apool.release()
        bpool.release()
        mtiles.release()
        acts.release()
        consts.release()
        dram.release()

    nc.compile()
    return nc


_NC_CACHE = {}


def _get_nc():
    stage = os.environ.get("KERNEL_STAGE", "full")
    if stage not in _NC_CACHE:
        _NC_CACHE[stage] = build_program(stage)
    return _NC_CACHE[stage]


def _make_in_maps(inputs):
    f = lambda a: np.ascontiguousarray(np.asarray(a, dtype=np.float32))
    x = f(inputs["x"])
    shared = {
        "W0": f(inputs["W0"]),
        "b0c": f(inputs["b0"]).reshape(HID, 1),
        "Wd0": f(inputs["Wd0"]),
        "bd0c": f(inputs["bd0"]).reshape(MB, 1),
        "beta0b": np.ascontiguousarray(
            np.broadcast_to(f(inputs["beta0"]), (P, CAT))
        ),
        "W1": f(inputs["W1"]),
        "b1c": f(inputs["b1"]).reshape(HID, 1),
        "Wd1": f(inputs["Wd1"]),
        "bd1c": f(inputs["bd1"]).reshape(MB, 1),
        "beta1b": np.ascontiguousarray(
            np.broadcast_to(f(inputs["beta1"]), (P, CAT))
        ),
        "Wfb": np.ascontiguousarray(
            np.broadcast_to(f(inputs["Wf"]).reshape(1, CAT), (P, CAT))
        ),
        "bfc": np.full((P, 1), float(np.asarray(inputs["bf"]).reshape(-1)[0]),
                       dtype=np.float32),
    }
    if BF16_NP is None:
        raise RuntimeError("ml_dtypes required for bf16 inputs")
    in_maps = []
    for c in range(NCORES):
        m = dict(shared)
        m["xT"] = np.ascontiguousarray(x[c * P : (c + 1) * P, :].T)
        sel = np.zeros((MB, NK), dtype=np.float32)
        for j in range(NK):
            sel[(10 * c + j) % MB, j] = 1.0
        m["Ssel"] = sel.astype(BF16_NP)
        in_maps.append(m)
    return in_maps


def run(inputs, **kw):
    nc = _get_nc()
    in_maps = _make_in_maps(inputs)
    res = run_bass_kernel_spmd(nc, in_maps, list(range(NCORES)), **kw)
    y = np.concatenate([res.results[c]["y"] for c in range(NCORES)], axis=0)
    return y.astype(np.float32), res


def kernel(**inputs) -> np.ndarray:
    y, _ = run(inputs)
    return y


# revision 94
# speedup vs baseline: 1.0576x; 1.0174x over previous
"""Trainium2 Bass kernel for nn_Discriminator (dense MLP + pairwise L1 diversity).

Sharding: data-parallel over N=1024 rows across 8 cores (128 rows each).
The tiny M = h @ Wd + bd ([N,100]) tensor is all-gathered (bf16) per block;
each core then computes its row-block of the N x N diversity reduction:

    div[j,k] = sum_i exp( - sum_d |M[i,k,d] - M[j,k,d]| )

Engine mapping per (k,d):
  - DMA broadcasts row kd of gathered M^T to a [128,1024] bf16 tile B
  - DVE tensor_scalar: A = |B - M_own[:,kd]|  (op0=subtract, op1=abs_max, 4x mode)
  - PE: identity matmul accumulates A over d into PSUM (l1, fp32)
  - ACT: one activation(Exp, scale=-1, accum_out=...) fuses exp + sum_i,
    writing the [128,1] result directly into the concat tile's column.
"""

import os
import sys

import numpy as np

sys.path.insert(0, "/opt/trn_rl_repo")

import concourse.bass as bass
import concourse.bacc as bacc
import concourse.tile as tile
from concourse import mybir
from concourse.bass_utils import run_bass_kernel_spmd

try:
    import ml_dtypes

    BF16_NP = ml_dtypes.bfloat16
except ImportError:  # pragma: no cover
    BF16_NP = None

F32 = mybir.dt.float32
BF16 = mybir.dt.bfloat16

N = 1024
NF = 512
HID = 256
NK = 10
KD = 10
MB = NK * KD  # 100
CAT = HID + NK  # 266
EPS = 1e-3
ALPHA = 0.3
NCORES = 8
P = N // NCORES  # 128 rows per core

AF = mybir.ActivationFunctionType
ALU = mybir.AluOpType


def _chunks(total, size):
    out = []
    o = 0
    while o < total:
        out.append((o, min(size, total - o)))
        o += size
    return out


def build_program(stage="full"):
    nc = bacc.Bacc(
        "TRN2",
        target_bir_lowering=False,
        debug=False,
        num_devices=NCORES,
    )

    # ---- per-core external inputs ----
    xT = nc.dram_tensor("xT", [NF, P], F32, kind="ExternalInput")
    W0 = nc.dram_tensor("W0", [NF, HID], F32, kind="ExternalInput")
    b0c = nc.dram_tensor("b0c", [HID, 1], F32, kind="ExternalInput")
    Wd0 = nc.dram_tensor("Wd0", [HID, MB], F32, kind="ExternalInput")
    bd0c = nc.dram_tensor("bd0c", [MB, 1], F32, kind="ExternalInput")
    beta0b = nc.dram_tensor("beta0b", [P, CAT], F32, kind="ExternalInput")
    W1 = nc.dram_tensor("W1", [CAT, HID], F32, kind="ExternalInput")
    b1c = nc.dram_tensor("b1c", [HID, 1], F32, kind="ExternalInput")
    Wd1 = nc.dram_tensor("Wd1", [HID, MB], F32, kind="ExternalInput")
    bd1c = nc.dram_tensor("bd1c", [MB, 1], F32, kind="ExternalInput")
    beta1b = nc.dram_tensor("beta1b", [P, CAT], F32, kind="ExternalInput")
    Wfb = nc.dram_tensor("Wfb", [P, CAT], F32, kind="ExternalInput")
    bfc = nc.dram_tensor("bfc", [P, 1], F32, kind="ExternalInput")

    # per-core one-hot [100, 10]: column m selects M^T row 10*core + m
    Ssel = nc.dram_tensor("Ssel", [MB, NK], BF16, kind="ExternalInput")

    y_out = nc.dram_tensor("y", [P, 1], F32, kind="ExternalOutput")

    # ---- NEFF-embedded constants ----
    ident_f32 = nc.inline_tensor(np.eye(128, dtype=np.float32), name="ident_f32")
    ident_bf16 = nc.inline_tensor(
        np.eye(128).astype(BF16_NP), name="ident_bf16"
    )
    ones1_f32 = nc.inline_tensor(
        np.ones((1, 128), dtype=np.float32), name="ones1_f32"
    )
    # column sums with -0.5 scaling for the Sb rows
    nh10_c = nc.inline_tensor(
        np.full((KD, 1), -0.5).astype(BF16_NP), name="nh10"
    )
    _nh2 = np.zeros((2 * KD, 2))
    _nh2[:KD, 0] = -0.5
    _nh2[KD:, 1] = -0.5
    nh20x2_c = nc.inline_tensor(_nh2.astype(BF16_NP), name="nh20x2")

    with tile.TileContext(nc, num_cores=NCORES) as tc:
        dram = tc.alloc_tile_pool(name="dram", bufs=1, space="DRAM")
        m_loc = [dram.tile([MB, P], BF16, name=f"m_loc{b}") for b in range(2)]
        m_gath = [
            dram.tile(
                [NCORES, MB, P], BF16,
                addr_space=("Local" if stage == "nocc" else "Shared"),
                name=f"m_gath{b}",
            )
            for b in range(2)
        ]
        # rows 80..99 of M^T (kernels 8, 9) and the selected kernel rows
        mt89_dram = [dram.tile([2 * KD, N], BF16, name=f"mt89_d{b}") for b in range(2)]
        mtA_dram = [dram.tile([KD, N], BF16, name=f"mtA_d{b}") for b in range(2)]
        # AllToAll of M^T rows 0..79: shard c = rows of kernel c, so every
        # core receives its own kernel's rows from all peers (1/10th the
        # AllGather payload, and off the mt_sb assembly path)
        mtam_recv = [
            dram.tile([NCORES, KD, P], BF16, name=f"mtam_r{b}") for b in range(2)
        ]
        a2a_send = [dram.tile([NCORES, P], F32, name=f"a2a_s{b}") for b in range(2)]
        a2a_recv = [
            dram.tile([NCORES, P], F32, name=f"a2a_r{b}") for b in range(2)
        ]
        consts = tc.alloc_tile_pool(name="consts", bufs=1)
        acts = tc.alloc_tile_pool(name="acts", bufs=1)
        mtiles = tc.alloc_tile_pool(name="mtiles", bufs=2)
        bpool = tc.alloc_tile_pool(name="bpool", bufs=2)
        apool = tc.alloc_tile_pool(name="apool", bufs=6)
        epool = tc.alloc_tile_pool(name="epool", bufs=2)
        rows = tc.alloc_tile_pool(name="rows", bufs=1)
        small = tc.alloc_tile_pool(name="small", bufs=4)
        ps_small = tc.alloc_tile_pool(name="ps_small", bufs=2, space="PSUM")
        ps_l1 = tc.alloc_tile_pool(name="ps_l1", bufs=3, space="PSUM")

        # ---------- load constants ----------
        # startup-critical consts via HWDGE (sync); only the late-needed
        # block-1/LN/head weights ride the Pool queue, few enough that the
        # M-chain DMAs queued behind them are not delayed
        def load(dram, shape, dtype=F32, name=None, late=False):
            t = consts.tile(shape, dtype, name=name)
            (nc.gpsimd if late else nc.sync).dma_start(out=t, in_=dram)
            return t

        xT_sb = [
            load(xT[o : o + sz, :], [sz, P], name=f"xT{i}")
            for i, (o, sz) in enumerate(_chunks(NF, 128))
        ]
        w0_sb = [
            load(W0[o : o + sz, :], [sz, HID], name=f"w0_{i}")
            for i, (o, sz) in enumerate(_chunks(NF, 128))
        ]
        idf = load(ident_f32[:, :], [128, 128], name="idf")
        idb = load(ident_bf16[:, :], [128, 128], BF16, name="idb")
        ones1 = load(ones1_f32[:, :], [1, 128], name="ones1")
        nh10 = load(nh10_c[:, :], [KD, 1], BF16, name="nh10")
        nh20x2 = load(nh20x2_c[:, :], [2 * KD, 2], BF16, name="nh20x2")
        w1_sb = [
            load(W1[o : o + sz, :], [sz, HID], name=f"w1_{i}", late=True)
            for i, (o, sz) in enumerate(_chunks(CAT, 128))
        ]
        wd0_sb = [
            load(Wd0[o : o + sz, :], [sz, MB], name=f"wd0_{i}")
            for i, (o, sz) in enumerate(_chunks(HID, 128))
        ]
        wd1_sb = [
            load(Wd1[o : o + sz, :], [sz, MB], name=f"wd1_{i}", late=True)
            for i, (o, sz) in enumerate(_chunks(HID, 128))
        ]
        b0_sb = [
            load(b0c[o : o + sz, :], [sz, 1], name=f"b0_{i}")
            for i, (o, sz) in enumerate(_chunks(HID, 128))
        ]
        b1_sb = [
            load(b1c[o : o + sz, :], [sz, 1], name=f"b1_{i}", late=True)
            for i, (o, sz) in enumerate(_chunks(HID, 128))
        ]
        bd0_sb = load(bd0c[:, :], [MB, 1], name="bd0")
        bd1_sb = load(bd1c[:, :], [MB, 1], name="bd1", late=True)
        beta_sb = [
            load(beta0b[:, :], [P, CAT], name="beta0", late=True),
            load(beta1b[:, :], [P, CAT], name="beta1", late=True),
        ]
        wf_sb = load(Wfb[:, :], [P, CAT], name="wf", late=True)
        bf_sb = load(bfc[:, :], [P, 1], name="bf", late=True)

        eps_sb = consts.tile([P, 1], F32, name="eps")
        nc.vector.memset(eps_sb, EPS)

        # ---------- one block ----------
        def block(b, prevT, w_sb, b_sb, wd_sb, bd_sb, do_div=True, upto=None):
            """prevT: list of (tile, psize) feature-major chunks of the input.

            Returns cat tile [P, CAT] = LeakyReLU(LN(concat(h, div))).
            """
            # h^T = W^T @ prev + b   (feature-major, HID x P as 2 chunks)
            hT = []
            for mi, (mo, msz) in enumerate(_chunks(HID, 128)):
                ps = ps_small.tile([128, P], F32, tag="ps_small")
                for ki, (wt, (pt, psz)) in enumerate(zip(w_sb, prevT)):
                    nc.tensor.matmul(
                        ps[:msz, :],
                        wt[:, mo : mo + msz],
                        pt,
                        start=(ki == 0),
                        stop=(ki == len(w_sb) - 1),
                    )
                ht = acts.tile([msz, P], F32, name=f"hT{b}_{mi}")
                nc.vector.tensor_scalar(
                    out=ht, in0=ps[:msz, :], scalar1=b_sb[mi], scalar2=None,
                    op0=ALU.add,
                )
                hT.append((ht, msz))
            if upto == "h":
                return hT[0][0]

            # M^T = Wd^T @ h + bd   [100, 128]
            ps_m = ps_small.tile([MB, P], F32, tag="ps_small")
            for ki, ((ht, _), wdt) in enumerate(zip(hT, wd_sb)):
                nc.tensor.matmul(
                    ps_m,
                    wdt,
                    ht,
                    start=(ki == 0),
                    stop=(ki == len(wd_sb) - 1),
                )
            mT = mtiles.tile([MB, P], F32, tag="mT")
            nc.vector.tensor_scalar(
                out=mT, in0=ps_m, scalar1=bd_sb, scalar2=None, op0=ALU.add
            )

            # own M rows (row-major, fp32) for per-partition scalars
            ps_t = ps_small.tile([128, MB], F32, tag="ps_small")
            nc.tensor.transpose(ps_t[:, :], mT, idf[:MB, :MB])
            m_row = mtiles.tile([P, MB], F32, tag="m_row")
            nc.vector.tensor_copy(m_row, ps_t[:, :MB])
            if upto == "m":
                return m_row

            # concat tile; div columns are filled by the diversity loop
            cat = acts.tile([P, CAT], F32, name=f"cat{b}")
            if not do_div:
                nc.vector.memset(cat[:, HID:CAT], 1.0)

            # ---- gather M^T and build per-core slices ----
            # unit u=0..7: (kernel = sel-core, J-block = u)
            # unit u=8, 9: (kernel 8/9, J-block = own rows)
            if do_div:
                # SWDGE casts f32 -> bf16 during the transfer; no DVE copy
                nc.gpsimd.dma_start(out=m_loc[b][:, :], in_=mT)
                if stage == "nocc":
                    nc.gpsimd.dma_start(
                        out=mtam_recv[b][:, :, :], in_=m_loc[b][0:80, :]
                    )
                    for c in range(NCORES):
                        nc.sync.dma_start(
                            out=m_gath[b][c, :, :], in_=m_loc[b][:, :]
                        )
                else:
                    nc.gpsimd.collective_compute(
                        "AllToAll",
                        ALU.bypass,
                        replica_groups=[list(range(NCORES))],
                        ins=[m_loc[b][0:80, :]],
                        outs=[mtam_recv[b][:, :, :]],
                    )
                    nc.gpsimd.collective_compute(
                        "AllGather",
                        ALU.bypass,
                        replica_groups=[list(range(NCORES))],
                        ins=[m_loc[b][:, :]],
                        outs=[m_gath[b][:, :, :]],
                    )
                # one DMA assembles [100, 1024] from the gathered blocks
                mt_sb = mtiles.tile([MB, N], BF16, tag="mt_sb")
                gsrc = m_gath[b][:, :, :]
                gath_ap = bass.AP(
                    tensor=gsrc.tensor,
                    offset=gsrc.offset,
                    ap=[[P, MB], [MB * P, NCORES], [1, P]],
                )
                nc.gpsimd.dma_start(out=mt_sb, in_=gath_ap)
                # kernels 8,9 rows -> DRAM (for broadcast) and base-0 SBUF
                nc.gpsimd.dma_start(out=mt89_dram[b][:, :], in_=mt_sb[80:100, :])
                mt89_sb = mtiles.tile([2 * KD, N], BF16, tag="mt89_sb")
                nc.gpsimd.dma_start(out=mt89_sb, in_=mt89_dram[b][:, :])
                # same 20 rows flattened onto partition 0 (partition_broadcast
                # sources must start at partition 0)
                mt89_row = rows.tile([1, 2 * KD * N], BF16, tag="mt89_row")
                nc.gpsimd.dma_start(
                    out=mt89_row,
                    in_=bass.AP(
                        tensor=mt89_dram[b][:, :].tensor,
                        offset=mt89_dram[b][:, :].offset,
                        ap=[[0, 1], [1, 2 * KD * N]],
                    ),
                )
                # own kernel's rows, assembled from the AllToAll result.
                # Two independent hops off the same source: DRAM->DRAM for the
                # broadcast source, DRAM->SBUF for negSb/scalars — parallel,
                # so the broadcast doesn't wait on the SBUF round-trip.
                rsrc = mtam_recv[b][:, :, :]
                asm_ap = bass.AP(
                    tensor=rsrc.tensor,
                    offset=rsrc.offset,
                    ap=[[P, KD], [KD * P, NCORES], [1, P]],
                )
                nc.gpsimd.dma_start(out=mtA_dram[b][:, :], in_=asm_ap)
                mtA_sb = mtiles.tile([KD, N], BF16, tag="mtA_sb")
                nc.gpsimd.dma_start(out=mtA_sb, in_=asm_ap)

                # -Sb/2 rows ([1, N] fp32 at partition 0) for the 3 kernels
                def sbrow(lhsT, rhs_sb, nm):
                    row = rows.tile([1, N], F32, tag=nm)
                    for ho, hsz in _chunks(N, 512):
                        ps_r = ps_small.tile([1, 512], F32, tag="ps_small")
                        nc.tensor.matmul(
                            ps_r[:, :hsz], lhsT, rhs_sb[:, ho : ho + hsz],
                            start=True, stop=True,
                        )
                        nc.scalar.activation(
                            row[:, ho : ho + hsz], ps_r[:, :hsz], AF.Copy,
                            bias=0.0, scale=1.0,
                        )
                    return row

                negsbA = sbrow(nh10, mtA_sb, "negsbA")
                negsb8 = sbrow(nh20x2[:, 0:1], mt89_sb, "negsb8")
                negsb9 = sbrow(nh20x2[:, 1:2], mt89_sb, "negsb9")

                # broadcast mega-tiles [128, 10*N]: same row set on every
                # partition (DMA reads the DRAM rows 128 times)
                def bmega(dram_ap, nm):
                    bt = bpool.tile([P, KD * N], BF16, tag="bt")
                    bcast = bass.AP(
                        tensor=dram_ap.tensor,
                        offset=dram_ap.offset,
                        ap=[[0, P], [1, KD * N]],
                    )
                    nc.gpsimd.dma_start(out=bt, in_=bcast)
                    return bt

                # split the broadcast so unit 0 can start after the first
                # two d-slices land instead of the full 2.5 MB
                btA0 = bpool.tile([P, 2 * N], BF16, tag="btA0")
                src0 = mtA_dram[b][0:1, :]
                nc.gpsimd.dma_start(
                    out=btA0,
                    in_=bass.AP(
                        tensor=src0.tensor, offset=src0.offset,
                        ap=[[0, P], [1, 2 * N]],
                    ),
                )
                btA1 = bpool.tile([P, (KD - 2) * N], BF16, tag="btA1")
                src1 = mtA_dram[b][2:3, :]
                nc.gpsimd.dma_start(
                    out=btA1,
                    in_=bass.AP(
                        tensor=src1.tensor, offset=src1.offset,
                        ap=[[0, P], [1, (KD - 2) * N]],
                    ),
                )

                # kernels 8/9: broadcast on the (otherwise idle) Pool engine
                # straight from SBUF; needed only at the end of the unit loop
                def bmega_pool(row0):
                    bt = bpool.tile([P, KD * N], BF16, tag="bt")
                    for d in range(KD):
                        nc.gpsimd.partition_broadcast(
                            bt[:, d * N : (d + 1) * N],
                            mt89_row[0:1, (row0 + d) * N : (row0 + d + 1) * N],
                        )
                    return bt

                bt8 = bmega_pool(0)
                bt9 = bmega_pool(KD)

                divsend = acts.tile([P, NCORES], F32, name=f"divsend{b}")

                for u in range(NK):
                    if u < NCORES:
                        negsb = negsbA

                        def bt_slice(d):
                            if d < 2:
                                return btA0[:, d * N : (d + 1) * N]
                            return btA1[:, (d - 2) * N : (d - 1) * N]
                        # scalars: M[J-block u rows, own-kernel cols] =
                        # transpose of the mtA slice for block u
                        ps_sc = ps_small.tile([128, KD], BF16, tag="ps_small")
                        nc.tensor.transpose(
                            ps_sc[:, :KD],
                            mtA_sb[:, u * P : (u + 1) * P],
                            idb[:KD, :KD],
                        )
                        scal = small.tile([P, KD], F32, tag="scal")
                        nc.vector.tensor_copy(scal, ps_sc[:, :KD])
                        accum_dst = divsend[:, u : u + 1]
                    else:
                        bt = bt8 if u == 8 else bt9
                        negsb = negsb8 if u == 8 else negsb9

                        def bt_slice(d, _bt=bt):
                            return _bt[:, d * N : (d + 1) * N]
                        scal = small.tile([P, KD], F32, tag="scal")
                        nc.vector.tensor_copy(
                            scal, m_row[:, (u - 8 + 8) * KD : (u - 7 + 8) * KD]
                        )
                        accum_dst = cat[:, HID + u : HID + u + 1]
                    nss = small.tile([P, 1], F32, tag="nss")
                    nc.vector.tensor_reduce(
                        out=nss, in_=scal, axis=mybir.AxisListType.X,
                        op=ALU.add, negate=True,
                    )
                    psl = ps_l1.tile([P, N], F32, tag="psl")

                    def relu_d(d):
                        at = apool.tile([P, N], BF16, tag="at")
                        nc.vector.tensor_scalar(
                            out=at,
                            in0=bt_slice(d),
                            scalar1=scal[:, d : d + 1],
                            scalar2=0.0,
                            op0=ALU.subtract,
                            op1=ALU.max,
                        )
                        return at

                    def stream(at, first):
                        for ho, hsz in _chunks(N, 512):
                            nc.tensor.matmul(
                                psl[:, ho : ho + hsz],
                                idb,
                                at[:, ho : ho + hsz],
                                start=first,
                                stop=False,
                            )

                    # d = 0..5 stream straight into PSUM; d = 6..9 are
                    # pre-added pairwise on DVE to offload the PE
                    for d in range(6):
                        stream(relu_d(d), d == 0)
                    for lo in (6, 8):
                        a0, a1 = relu_d(lo), relu_d(lo + 1)
                        comb = apool.tile([P, N], BF16, tag="comb")
                        nc.vector.tensor_add(comb, a0, a1)
                        stream(comb, False)
                    for ho, hsz in _chunks(N, 512):
                        nc.tensor.matmul(
                            psl[:, ho : ho + hsz],
                            ones1,
                            negsb[:, ho : ho + hsz],
                            start=False,
                            stop=True,
                        )
                    escr = epool.tile([P, N], BF16, tag="escr")
                    nc.scalar.activation(
                        escr, psl, AF.Exp, bias=nss, scale=-2.0,
                        accum_out=accum_dst,
                    )

                # exchange div columns: shard u of our send buffer holds the
                # result for core u; AllToAll routes sender k's shard c to
                # slot k on core c  ->  recv[k] = div[own rows, kernel k]
                ps_ds = ps_small.tile([128, P], F32, tag="ps_small")
                nc.tensor.transpose(ps_ds[:NCORES, :], divsend, idf)
                dsend_sb = small.tile([NCORES, P], F32, tag="dsend")
                nc.vector.tensor_copy(dsend_sb, ps_ds[:NCORES, :])
                nc.gpsimd.dma_start(out=a2a_send[b][:, :], in_=dsend_sb)
                if stage == "nocc":
                    nc.gpsimd.dma_start(
                        out=a2a_recv[b][:, :], in_=a2a_send[b][:, :]
                    )
                else:
                    nc.gpsimd.collective_compute(
                        "AllToAll",
                        ALU.bypass,
                        replica_groups=[list(range(NCORES))],
                        ins=[a2a_send[b][:, :]],
                        outs=[a2a_recv[b][:, :]],
                    )
                drecv_sb = small.tile([NCORES, P], F32, tag="drecv")
                nc.gpsimd.dma_start(out=drecv_sb, in_=a2a_recv[b][:, :])
                ps_dr = ps_small.tile([128, NCORES], F32, tag="ps_small")
                nc.tensor.transpose(
                    ps_dr[:, :NCORES], drecv_sb, idf[:NCORES, :NCORES]
                )
                nc.vector.tensor_copy(
                    cat[:, HID : HID + NCORES], ps_dr[:, :NCORES]
                )

            # h rows into cat[:, :256] via PE transposes of hT
            for mi, (ht, msz) in enumerate(hT):
                ps_t2 = ps_small.tile([128, P], F32, tag="ps_small")
                nc.tensor.transpose(ps_t2[:, :msz], ht, idf[:msz, :msz])
                nc.vector.tensor_copy(
                    cat[:, mi * 128 : mi * 128 + msz], ps_t2[:, :msz]
                )

            if upto == "cat":
                return cat
            # LayerNorm (center+scale, beta only) + LeakyReLU
            stats = small.tile([P, 6], F32, tag="stats")
            nc.vector.bn_stats(out=stats, in_=cat)
            mv = small.tile([P, 2], F32, tag="mv")
            nc.vector.bn_aggr(out=mv, in_=stats)
            rstd = small.tile([P, 1], F32, tag="rstd")
            nc.scalar.activation(
                rstd, mv[:, 1:2], AF.Sqrt, bias=eps_sb, scale=1.0
            )
            nc.vector.reciprocal(out=rstd, in_=rstd)
            if upto == "stats":
                return mv
            catn = acts.tile([P, CAT], F32, name=f"catn{b}")
            nc.vector.tensor_scalar(
                out=catn,
                in0=cat,
                scalar1=mv[:, 0:1],
                scalar2=rstd,
                op0=ALU.subtract,
                op1=ALU.mult,
            )
            nc.vector.tensor_add(catn, catn, beta_sb[b])
            if upto == "ln":
                return catn
            # leaky relu: max(x, 0.3x)
            scr = acts.tile([P, CAT], F32, name=f"lrelu{b}")
            nc.scalar.activation(scr, catn, AF.Copy, bias=0.0, scale=ALPHA)
            hout = acts.tile([P, CAT], F32, name=f"hout{b}")
            nc.vector.tensor_tensor(
                out=hout, in0=catn, in1=scr, op=ALU.max
            )
            if upto == "lrelu":
                return hout
            return hout

        # ---------- block 0 ----------
        prev0 = [(t, 128) for t in xT_sb]
        upto = stage if stage in ("h", "m", "cat", "stats", "ln", "lrelu") else None
        h1 = block(0, prev0, w0_sb, b0_sb, wd0_sb, bd0_sb,
                   do_div=(stage in ("full", "b0", "nocc")), upto=upto)
        if upto is not None:
            ytmp = small.tile([P, 1], F32, tag="ysb")
            nc.vector.tensor_copy(ytmp, h1[:, 0:1])
            nc.sync.dma_start(out=y_out[:, :], in_=ytmp)
            h1 = None

        if upto is not None:
            pass
        elif stage in ("full", "nocc"):
            # transpose h1 -> feature-major chunks for block 1
            h1T = []
            for ci, (co, csz) in enumerate(_chunks(CAT, 128)):
                ps_t = ps_small.tile([128, P], F32, tag="ps_small")
                nc.tensor.transpose(ps_t[:csz, :], h1[:, co : co + csz], idf)
                ht = acts.tile([csz, P], F32, name=f"h1T_{ci}")
                nc.vector.tensor_copy(ht, ps_t[:csz, :])
                h1T.append((ht, csz))

            # ---------- block 1 ----------
            h2 = block(1, h1T, w1_sb, b1_sb, wd1_sb, bd1_sb)
        else:
            h2 = h1

        # ---------- critic head: y = h2 @ Wf + bf ----------
        if upto is None:
            hw = acts.tile([P, CAT], F32, name="hw")
            yacc = small.tile([P, 1], F32, tag="yacc")
            nc.vector.tensor_mul(hw, h2, wf_sb)
            nc.vector.tensor_reduce(
                out=yacc, in_=hw, axis=mybir.AxisListType.X, op=ALU.add
            )
            ysb = small.tile([P, 1], F32, tag="ysb")
            nc.scalar.activation(ysb, yacc, AF.Identity, bias=bf_sb, scale=1.0)
            nc.sync.dma_start(out=y_out[:, :], in_=ysb)

        ps_l1.release()
        ps_small.release()
        small.release()
        rows.release()
        epool.release()
        apool.release()
        bpool.release()
        mtiles.release()
        acts.release()
        consts.release()
        dram.release()

    nc.compile()
    return nc


_NC_CACHE = {}


def _get_nc():
    stage = os.environ.get("KERNEL_STAGE", "full")
    if stage not in _NC_CACHE:
        _NC_CACHE[stage] = build_program(stage)
    return _NC_CACHE[stage]


def _make_in_maps(inputs):
    f = lambda a: np.ascontiguousarray(np.asarray(a, dtype=np.float32))
    x = f(inputs["x"])
    shared = {
        "W0": f(inputs["W0"]),
        "b0c": f(inputs["b0"]).reshape(HID, 1),
        "Wd0": f(inputs["Wd0"]),
        "bd0c": f(inputs["bd0"]).reshape(MB, 1),
        "beta0b": np.ascontiguousarray(
            np.broadcast_to(f(inputs["beta0"]), (P, CAT))
        ),
        "W1": f(inputs["W1"]),
        "b1c": f(inputs["b1"]).reshape(HID, 1),
        "Wd1": f(inputs["Wd1"]),
        "bd1c": f(inputs["bd1"]).reshape(MB, 1),
        "beta1b": np.ascontiguousarray(
            np.broadcast_to(f(inputs["beta1"]), (P, CAT))
        ),
        "Wfb": np.ascontiguousarray(
            np.broadcast_to(f(inputs["Wf"]).reshape(1, CAT), (P, CAT))
        ),
        "bfc": np.full((P, 1), float(np.asarray(inputs["bf"]).reshape(-1)[0]),
                       dtype=np.float32),
    }
    if BF16_NP is None:
        raise RuntimeError("ml_dtypes required for bf16 inputs")
    in_maps = []
    for c in range(NCORES):
        m = dict(shared)
        m["xT"] = np.ascontiguousarray(x[c * P : (c + 1) * P, :].T)
        sel = np.zeros((MB, NK), dtype=np.float32)
        for j in range(NK):
            sel[(10 * c + j) % MB, j] = 1.0
        m["Ssel"] = sel.astype(BF16_NP)
        in_maps.append(m)
    return in_maps


def run(inputs, **kw):
    nc = _get_nc()
    in_maps = _make_in_maps(inputs)
    res = run_bass_kernel_spmd(nc, in_maps, list(range(NCORES)), **kw)
    y = np.concatenate([res.results[c]["y"] for c in range(NCORES)], axis=0)
    return y.astype(np.float32), res


def kernel(**inputs) -> np.ndarray:
    y, _ = run(inputs)
    return y
